# revision 10
# baseline (speedup 1.0000x reference)
"""GemmaAttention (B=1, S=2048, H=2048, NH=16, HD=128) on 8 trn2 NeuronCores.

Sharding: tensor-parallel over heads -- core c owns heads {2c, 2c+1}.
Each core projects q/k/v for its 2 heads (contraction over H=2048 with
host-pre-transposed activations so the contraction dim is on partitions),
applies RoPE (the reference's rotate_half is an identity concat, so RoPE
collapses to an elementwise multiply by cos+sin), computes scores both
ways (S = qT'k and S^T = kT'q, so softmax stats come from the natural
layout while the attn@v matmul gets the transposed operand without an
explicit transpose), does softmax without max-subtraction (scores are
O(1) for this problem; exp stays far from overflow), and computes its
partial o_proj output transposed.  Host sums the 8 partial o_proj
outputs and stacks the per-core attention-probability slabs.
"""

import sys
import types

sys.path.insert(0, "/opt/trn_rl_repo")

import numpy as np
import ml_dtypes

P = 128
S = 2048
H = 2048
HD = 128
NH = 16
N_CORES = 8
HPC = 2  # heads per core
DPC = HPC * HD  # 256 projection dims per core
GAMMA = float(HD) ** -0.5
BF16 = ml_dtypes.bfloat16

_CACHE = {}


def _install_ntff_hook():
    """bass_utils' trace path imports antenv.axon_hooks, which this image
    lacks; synthesize it and register the ctypes-based NTFF profile hook."""
    if "antenv.axon_hooks" in sys.modules:
        return
    import antenv

    mod = types.ModuleType("antenv.axon_hooks")
    holder = [None]
    mod.set_axon_ntff_profile_hook = lambda h: holder.__setitem__(0, h)
    mod.get_axon_ntff_profile_hook = lambda: holder[0]
    sys.modules["antenv.axon_hooks"] = mod
    antenv.axon_hooks = mod
    try:
        from trn_agent_boot.trn_boot import _ntff_profile_via_ctypes

        mod.set_axon_ntff_profile_hook(
            _ntff_profile_via_ctypes("/opt/axon/libaxon_pjrt.so")
        )
    except Exception:
        pass


def _build_nc(phase="full"):
    import concourse.bass as bass
    from concourse import bacc
    import concourse.mybir as mybir
    import concourse.tile as tile
    from concourse.masks import make_identity
    from contextlib import ExitStack

    f32 = mybir.dt.float32
    bf16 = mybir.dt.bfloat16
    mult = mybir.AluOpType.mult
    Exp = mybir.ActivationFunctionType.Exp

    nc = bacc.Bacc("TRN2", target_bir_lowering=False, debug=False)

    xqT = nc.dram_tensor("xqT", [H, S], bf16, kind="ExternalInput")
    kxT = nc.dram_tensor("kxT", [H, S], bf16, kind="ExternalInput")
    csT = nc.dram_tensor("csT", [HD, S], f32, kind="ExternalInput")
    wkt = nc.dram_tensor("wkt", [H, DPC], bf16, kind="ExternalInput")  # q proj
    wqt = nc.dram_tensor("wqt", [H, DPC], bf16, kind="ExternalInput")  # k proj
    wvt = nc.dram_tensor("wvt", [H, DPC], bf16, kind="ExternalInput")
    wot = nc.dram_tensor("wot", [DPC, H], bf16, kind="ExternalInput")
    outT = nc.dram_tensor("outT", [H, S], f32, kind="ExternalOutput")
    attnw = nc.dram_tensor("attnw", [HPC, S, S], f32, kind="ExternalOutput")

    NC16 = H // P  # 16 contraction tiles
    NT = S // P  # 16 row tiles
    NJ4 = S // 512  # 4 free-dim chunks

    with tile.TileContext(nc) as tc, ExitStack() as ctx:
        psum = ctx.enter_context(tc.tile_pool(name="psum", bufs=2, space="PSUM"))
        sb = ctx.enter_context(tc.tile_pool(name="sb", bufs=1))

        # ---- constants & weights --------------------------------------
        cs_sb = sb.tile([HD, S], f32, tag="cs", bufs=1)
        nc.sync.dma_start(cs_sb[:], csT[:])
        ident = sb.tile([P, P], f32, tag="id", bufs=1)
        make_identity(nc, ident)
        id16 = sb.tile([16, 16], f32, tag="id16", bufs=1)
        make_identity(nc, id16)

        w_sb = {}
        for nm, drm in (("wk", wkt), ("wq", wqt), ("wv", wvt)):
            tiles = []
            for c in range(NC16):
                t = sb.tile([P, DPC], bf16, tag=nm, bufs=NC16, name=f"{nm}{c}")
                nc.sync.dma_start(t[:], drm[c * P : (c + 1) * P, :])
                tiles.append(t)
            w_sb[nm] = tiles
        wo_sb = []
        for d in range(HPC):
            t = sb.tile([P, H], bf16, tag="wo", bufs=HPC, name=f"wo{d}")
            nc.sync.dma_start(t[:], wot[d * P : (d + 1) * P, :])
            wo_sb.append(t)

        # ---- phase A: projections -------------------------------------
        def load_acts(drm, nm):
            tiles = []
            for c in range(NC16):
                t = sb.tile([P, S], bf16, tag="act", bufs=NC16, name=f"{nm}{c}")
                nc.sync.dma_start(t[:], drm[c * P : (c + 1) * P, :])
                tiles.append(t)
            return tiles

        def proj(x_tiles, w_tiles, rope):
            """Returns 2 bf16 [P, S] tiles (one per head's 128 dims)."""
            quads = [
                psum.tile([P, S], f32, tag="quad", name=f"pq{d}") for d in range(HPC)
            ]
            for d in range(HPC):
                for s4 in range(NJ4):
                    for c in range(NC16):
                        nc.tensor.matmul(
                            quads[d][:, s4 * 512 : (s4 + 1) * 512],
                            lhsT=w_tiles[c][:, d * P : (d + 1) * P],
                            rhs=x_tiles[c][:, s4 * 512 : (s4 + 1) * 512],
                            start=(c == 0),
                            stop=(c == NC16 - 1),
                        )
            outs = []
            for d in range(HPC):
                o = sb.tile([P, S], bf16, tag="qk", bufs=4, name=f"prj{d}")
                if rope:
                    nc.vector.tensor_tensor(o[:], quads[d][:], cs_sb[:], op=mult)
                else:
                    nc.vector.tensor_copy(o[:], quads[d][:])
                outs.append(o)
            return outs

        def dbg_out(src_ap, row0):
            d = sb.tile([P, S], f32, tag="probs", bufs=3, name=f"dbg{row0}")
            nc.vector.tensor_copy(d[:], src_ap)
            nc.sync.dma_start(outT[row0 * P : (row0 + 1) * P, :], d[:])

        x_tiles = load_acts(xqT, "xe")
        qT = proj(x_tiles, w_sb["wk"], rope=True)  # ref swaps: Wk acts on queries
        kx_tiles = load_acts(kxT, "ke")
        kT = proj(kx_tiles, w_sb["wq"], rope=True)  # ... and Wq on keys

        if phase == "proj":
            dbg_out(qT[0][:], 0)
            dbg_out(qT[1][:], 1)
            dbg_out(kT[0][:], 2)
            dbg_out(kT[1][:], 3)

        # v: [j, d] layout (keys on partitions).  Two rounds of 8 j-blocks;
        # each j-block gets its own psum bank (256 used of 512) to keep one
        # accumulation group per zero-region.
        vw = []
        v_rounds = 2 if phase != "proj" else 0
        for r in range(v_rounds):
            t = sb.tile([P, S], bf16, tag="vw", bufs=2, name=f"vw{r}")
            vw.append(t)
        for r in range(v_rounds):
            vq = [
                psum.tile([P, S], f32, tag="quad", name=f"vq{r}{i}") for i in range(2)
            ]
            for jj in range(8):
                j = r * 8 + jj
                q_i, b = divmod(jj, 4)
                for c in range(NC16):
                    nc.tensor.matmul(
                        vq[q_i][:, b * 512 : b * 512 + DPC],
                        lhsT=kx_tiles[c][:, j * P : (j + 1) * P],
                        rhs=w_sb["wv"][c][:],
                        start=(c == 0),
                        stop=(c == NC16 - 1),
                    )
            for q_i in range(2):
                src = vq[q_i].rearrange("p (b n) -> p b n", n=512)[:, :, :DPC]
                dst = vw[r][:, q_i * 1024 : (q_i + 1) * 1024].rearrange(
                    "p (b n) -> p b n", n=DPC
                )
                nc.scalar.copy(dst, src)

        if phase == "v":
            dbg_out(vw[0][:], 0)
            dbg_out(vw[1][:], 1)

        # ---- phase B: attention per head ------------------------------
        n_heads = HPC if phase in ("full", "B", "B1", "B1r") else 0
        aT = []
        for h in range(n_heads):
            sums = sb.tile([P, NT], f32, tag="sums", bufs=2, name=f"sums{h}")
            rmat = sb.tile([P, NT], f32, tag="rmat", bufs=2, name=f"rmat{h}")
            # B1: scores [i, j], softmax stats + fp32 probs out
            for t in range(NT):
                scq = psum.tile([P, S], f32, tag="quad", name=f"scq{h}_{t}")
                for j4 in range(NJ4):
                    nc.tensor.matmul(
                        scq[:, j4 * 512 : (j4 + 1) * 512],
                        lhsT=qT[h][:, t * P : (t + 1) * P],
                        rhs=kT[h][:, j4 * 512 : (j4 + 1) * 512],
                        start=True,
                        stop=True,
                    )
                probs = sb.tile([P, S], f32, tag="probs", bufs=3, name=f"pr{h}_{t}")
                nc.scalar.activation(
                    probs[:], scq[:], Exp, scale=GAMMA, accum_out=sums[:, t : t + 1]
                )
                nc.vector.reciprocal(rmat[:, t : t + 1], sums[:, t : t + 1])
                nc.vector.tensor_scalar_mul(probs[:], probs[:], rmat[:, t : t + 1])
                nc.sync.dma_start(attnw[h, t * P : (t + 1) * P, :], probs[:])

            if phase == "B1":
                continue
            # r broadcast: rmat [128i, 16t] -> row-major [1, 2048] replicated
            # across partitions, via PE transpose + one-hot-row matmuls.
            rT_ps = psum.tile([NT, P], f32, tag="quad", name=f"rT{h}")
            nc.tensor.transpose(rT_ps[:], rmat[:], ident[:])
            rT_sb = sb.tile([NT, P], f32, tag="rT", bufs=2, name=f"rTs{h}")
            nc.vector.tensor_copy(rT_sb[:], rT_ps[:])
            rbq = psum.tile([P, S], f32, tag="quad", name=f"rbq{h}")
            for t in range(NT):
                nc.tensor.matmul(
                    rbq[:, t * P : (t + 1) * P],
                    lhsT=id16[:, t : t + 1].broadcast_to([NT, P]),
                    rhs=rT_sb[:],
                    start=True,
                    stop=True,
                )
            rbc = sb.tile([P, S], f32, tag="rbc", bufs=1, name=f"rbc{h}")
            nc.vector.tensor_copy(rbc[:], rbq[:])
            if phase == "B1r":
                dbg_out(rbc[:], h)
                continue

            # B2/B3: scores^T j-tiles -> exp -> attn@v accumulation
            attq = psum.tile([P, S], f32, tag="quad", name=f"attq{h}")
            for j in range(NT):
                stq = psum.tile([P, S], f32, tag="quad", name=f"stq{h}_{j}")
                for i4 in range(NJ4):
                    nc.tensor.matmul(
                        stq[:, i4 * 512 : (i4 + 1) * 512],
                        lhsT=kT[h][:, j * P : (j + 1) * P],
                        rhs=qT[h][:, i4 * 512 : (i4 + 1) * 512],
                        start=True,
                        stop=True,
                    )
                expT = sb.tile([P, S], bf16, tag="act", bufs=NC16, name=f"eT{h}_{j}")
                nc.scalar.activation(expT[:], stq[:], Exp, scale=GAMMA)
                r, jj = divmod(j, 8)
                for i4 in range(NJ4):
                    nc.tensor.matmul(
                        attq[:, i4 * 512 : (i4 + 1) * 512],
                        lhsT=vw[r][:, jj * DPC + h * P : jj * DPC + (h + 1) * P],
                        rhs=expT[:, i4 * 512 : (i4 + 1) * 512],
                        start=(j == 0),
                        stop=(j == NT - 1),
                    )
            a = sb.tile([P, S], bf16, tag="aT", bufs=2, name=f"aT{h}")
            nc.vector.tensor_tensor(a[:], attq[:], rbc[:], op=mult)
            aT.append(a)

        if phase == "B":
            dbg_out(aT[0][:], 0)
            dbg_out(aT[1][:], 1)

        # ---- phase C: o_proj (partial; host sums across cores) --------
        for og in range(NT if phase == "full" else 0):
            opq = psum.tile([P, S], f32, tag="quad", name=f"opq{og}")
            for i4 in range(NJ4):
                for d in range(HPC):
                    nc.tensor.matmul(
                        opq[:, i4 * 512 : (i4 + 1) * 512],
                        lhsT=wo_sb[d][:, og * P : (og + 1) * P],
                        rhs=aT[d][:, i4 * 512 : (i4 + 1) * 512],
                        start=(d == 0),
                        stop=(d == HPC - 1),
                    )
            oev = sb.tile([P, S], f32, tag="probs", bufs=3, name=f"oev{og}")
            if og % 2 == 0:
                nc.scalar.copy(oev[:], opq[:])
            else:
                nc.vector.tensor_copy(oev[:], opq[:])
            nc.sync.dma_start(outT[og * P : (og + 1) * P, :], oev[:])

    nc.compile()
    return nc


def _get_nc():
    if "nc" not in _CACHE:
        _install_ntff_hook()
        _CACHE["nc"] = _build_nc()
    return _CACHE["nc"]


def kernel(query_states, key_states, cos, sin, Wq, Wk, Wv, Wo, _trace=False):
    from concourse.bass_utils import run_bass_kernel_spmd

    nc = _get_nc()

    x = np.asarray(query_states, dtype=np.float32)[0]
    kx = np.asarray(key_states, dtype=np.float32)[0]
    cs = np.asarray(cos, dtype=np.float32)[0] + np.asarray(sin, dtype=np.float32)[0]
    xqT = x.T.astype(BF16)
    kxT_a = kx.T.astype(BF16)
    csT = np.ascontiguousarray(cs.T)
    Wq_, Wk_, Wv_, Wo_ = (np.asarray(w, dtype=np.float32) for w in (Wq, Wk, Wv, Wo))

    in_maps = []
    for c in range(N_CORES):
        sl = slice(c * DPC, (c + 1) * DPC)
        in_maps.append(
            {
                "xqT": xqT,
                "kxT": kxT_a,
                "csT": csT,
                "wkt": Wk_[sl, :].T.astype(BF16),
                "wqt": Wq_[sl, :].T.astype(BF16),
                "wvt": Wv_[sl, :].T.astype(BF16),
                "wot": Wo_[:, sl].T.astype(BF16),
            }
        )

    res = run_bass_kernel_spmd(nc, in_maps, list(range(N_CORES)), trace=_trace)
    _CACHE["last_results"] = res

    outT = res.results[0]["outT"].astype(np.float64)
    for c in range(1, N_CORES):
        outT += res.results[c]["outT"]
    out = np.ascontiguousarray(outT.T).astype(np.float32).reshape(1, S, H)
    attn = np.stack([res.results[c]["attnw"] for c in range(N_CORES)])
    attn = attn.reshape(1, NH, S, S)
    return out, attn


# revision 11
# speedup vs baseline: 1.5344x; 1.5344x over previous
"""GemmaAttention (B=1, S=2048, H=2048, NH=16, HD=128) on 8 trn2 NeuronCores.

Sharding: tensor-parallel over heads -- core c owns heads {2c, 2c+1}.
Each core projects q/k/v for its 2 heads (contraction over H=2048 with
host-pre-transposed activations so the contraction dim is on partitions),
applies RoPE (the reference's rotate_half is an identity concat, so RoPE
collapses to an elementwise multiply by cos+sin), computes scores both
ways (S = qT'k and S^T = kT'q, so softmax stats come from the natural
layout while the attn@v matmul gets the transposed operand without an
explicit transpose), does softmax without max-subtraction (scores are
O(1) for this problem; exp stays far from overflow), and computes its
partial o_proj output transposed.  Host sums the 8 partial o_proj
outputs and stacks the per-core attention-probability slabs.

Pipelining: all PSUM goes through one [128, 1024] half-bank-pair tag
(4 bufs = 8 banks).  Projection matmuls run contraction-outer so the PE
rides the input DMA stream; the v projection overlaps head-0's softmax;
head-1's scores/softmax overlap head-0's attn@v loop.
"""

import sys
import types

sys.path.insert(0, "/opt/trn_rl_repo")

import numpy as np
import ml_dtypes

P = 128
S = 2048
H = 2048
HD = 128
NH = 16
N_CORES = 8
HPC = 2  # heads per core
DPC = HPC * HD  # 256 projection dims per core
GAMMA = float(HD) ** -0.5
BF16 = ml_dtypes.bfloat16

_CACHE = {}


def _install_ntff_hook():
    """bass_utils' trace path imports antenv.axon_hooks, which this image
    lacks; synthesize it and register the ctypes-based NTFF profile hook."""
    if "antenv.axon_hooks" in sys.modules:
        return
    import antenv

    mod = types.ModuleType("antenv.axon_hooks")
    holder = [None]
    mod.set_axon_ntff_profile_hook = lambda h: holder.__setitem__(0, h)
    mod.get_axon_ntff_profile_hook = lambda: holder[0]
    sys.modules["antenv.axon_hooks"] = mod
    antenv.axon_hooks = mod
    try:
        from trn_agent_boot.trn_boot import _ntff_profile_via_ctypes

        mod.set_axon_ntff_profile_hook(
            _ntff_profile_via_ctypes("/opt/axon/libaxon_pjrt.so")
        )
    except Exception:
        pass


def _build_nc():
    import concourse.bass as bass
    from concourse import bacc
    import concourse.mybir as mybir
    import concourse.tile as tile
    from concourse.masks import make_identity
    from contextlib import ExitStack

    f32 = mybir.dt.float32
    bf16 = mybir.dt.bfloat16
    mult = mybir.AluOpType.mult
    Exp = mybir.ActivationFunctionType.Exp

    nc = bacc.Bacc("TRN2", target_bir_lowering=False, debug=False)

    xqT = nc.dram_tensor("xqT", [H, S], bf16, kind="ExternalInput")
    kxT = nc.dram_tensor("kxT", [H, S], bf16, kind="ExternalInput")
    csT = nc.dram_tensor("csT", [HD, S], f32, kind="ExternalInput")
    wkt = nc.dram_tensor("wkt", [H, DPC], bf16, kind="ExternalInput")  # q proj
    wqt = nc.dram_tensor("wqt", [H, DPC], bf16, kind="ExternalInput")  # k proj
    wvt = nc.dram_tensor("wvt", [H, DPC], bf16, kind="ExternalInput")
    wot = nc.dram_tensor("wot", [DPC, H], bf16, kind="ExternalInput")
    outT = nc.dram_tensor("outT", [H, S], f32, kind="ExternalOutput")
    attnw = nc.dram_tensor("attnw", [HPC, S, S], f32, kind="ExternalOutput")

    NC16 = H // P  # 16 contraction tiles
    NT = S // P  # 16 row tiles
    HF = 1024  # psum half-tile free size

    with tile.TileContext(nc) as tc, ExitStack() as ctx:
        psum = ctx.enter_context(tc.tile_pool(name="psum", bufs=4, space="PSUM"))
        sb = ctx.enter_context(tc.tile_pool(name="sb", bufs=1))

        def half(name):
            return psum.tile([P, HF], f32, tag="half", name=name)

        # ---- constants + q-pass inputs (interleaved for fast PE start) --
        wk_sb, xe = [], []
        for c in range(NC16):
            t = sb.tile([P, DPC], bf16, tag="wk", bufs=NC16, name=f"wk{c}")
            nc.sync.dma_start(t[:], wkt[c * P : (c + 1) * P, :])
            wk_sb.append(t)
            t = sb.tile([P, S], bf16, tag="xe", bufs=4, name=f"xe{c}")
            nc.sync.dma_start(t[:], xqT[c * P : (c + 1) * P, :])
            xe.append(t)

        # ---- q projection: c-outer, PE rides the DMA stream ------------
        def proj_mms(x_tiles, w_tiles):
            halves = [half(f"pj{i}") for i in range(4)]  # (d, lo/hi)
            for c in range(NC16):
                for d in range(HPC):
                    for s4 in range(4):
                        nc.tensor.matmul(
                            halves[d * 2 + s4 // 2][:, (s4 % 2) * 512 : (s4 % 2 + 1) * 512],
                            lhsT=w_tiles[c][:, d * P : (d + 1) * P],
                            rhs=x_tiles[c][:, s4 * 512 : (s4 + 1) * 512],
                            start=(c == 0),
                            stop=(c == NC16 - 1),
                        )
            return halves

        q_halves = proj_mms(xe, wk_sb)

        cs_sb = sb.tile([HD, S], f32, tag="cs", bufs=1)
        nc.sync.dma_start(cs_sb[:], csT[:])
        ident = sb.tile([P, P], f32, tag="id", bufs=1)
        make_identity(nc, ident)
        id16 = sb.tile([16, 16], f32, tag="id16", bufs=1)
        make_identity(nc, id16)

        # k-pass inputs behind q's
        wq_sb, ke = [], []
        for c in range(NC16):
            t = sb.tile([P, DPC], bf16, tag="wq", bufs=NC16, name=f"wq{c}")
            nc.sync.dma_start(t[:], wqt[c * P : (c + 1) * P, :])
            wq_sb.append(t)
            t = sb.tile([P, S], bf16, tag="ke", bufs=NC16, name=f"ke{c}")
            nc.sync.dma_start(t[:], kxT[c * P : (c + 1) * P, :])
            ke.append(t)

        def rope_evict(halves, nm):
            outs = []
            for d in range(HPC):
                o = sb.tile([P, S], bf16, tag="qk", bufs=4, name=f"{nm}{d}")
                for i in range(2):
                    nc.vector.tensor_tensor(
                        o[:, i * HF : (i + 1) * HF],
                        halves[d * 2 + i][:],
                        cs_sb[:, i * HF : (i + 1) * HF],
                        op=mult,
                    )
                outs.append(o)
            return outs

        qT = rope_evict(q_halves, "qT")  # ref swaps: Wk acts on queries
        k_halves = proj_mms(ke, wq_sb)
        kT = rope_evict(k_halves, "kT")  # ... and Wq acts on keys

        wv_sb = []
        for c in range(NC16):
            t = sb.tile([P, DPC], bf16, tag="wv", bufs=NC16, name=f"wv{c}")
            nc.sync.dma_start(t[:], wvt[c * P : (c + 1) * P, :])
            wv_sb.append(t)
        wo_sb = []
        for d in range(HPC):
            t = sb.tile([P, H], bf16, tag="wo", bufs=HPC, name=f"wo{d}")
            nc.sync.dma_start(t[:], wot[d * P : (d + 1) * P, :])
            wo_sb.append(t)

        vw = [
            sb.tile([P, S], bf16, tag="vw", bufs=2, name=f"vw{r}") for r in range(2)
        ]

        def v_round(r):
            """j-blocks 4r..4r+3 -> vw[r//2][:, (r%2)*1024:...]. Two halves."""
            vh = [half(f"v{r}_{i}") for i in range(2)]
            for c in range(NC16):
                for jj in range(4):
                    j = r * 4 + jj
                    nc.tensor.matmul(
                        vh[jj // 2][:, (jj % 2) * 512 : (jj % 2) * 512 + DPC],
                        lhsT=ke[c][:, j * P : (j + 1) * P],
                        rhs=wv_sb[c][:],
                        start=(c == 0),
                        stop=(c == NC16 - 1),
                    )
            for i in range(2):
                src = vh[i].rearrange("p (b n) -> p b n", n=512)[:, :, :DPC]
                dst = vw[r // 2][
                    :, (r % 2) * HF + i * 512 : (r % 2) * HF + (i + 1) * 512
                ].rearrange("p (b n) -> p b n", n=DPC)
                nc.scalar.copy(dst, src)

        # ---- phase B helpers -------------------------------------------
        sums = [
            sb.tile([P, NT], f32, tag="sums", bufs=2, name=f"sums{h}")
            for h in range(HPC)
        ]
        rmat = [
            sb.tile([P, NT], f32, tag="rmat", bufs=2, name=f"rmat{h}")
            for h in range(HPC)
        ]

        def b1_tile(h, t):
            """scores[i,j] for i-tile t, exp+sums, normalize, DMA probs."""
            sc = [half(f"sc{h}_{t}_{i}") for i in range(2)]
            for j4 in range(4):
                nc.tensor.matmul(
                    sc[j4 // 2][:, (j4 % 2) * 512 : (j4 % 2 + 1) * 512],
                    lhsT=qT[h][:, t * P : (t + 1) * P],
                    rhs=kT[h][:, j4 * 512 : (j4 + 1) * 512],
                    start=True,
                    stop=True,
                )
            probs = sb.tile([P, S], f32, tag="probs", bufs=3, name=f"pr{h}_{t}")
            s2 = sb.tile([P, 2], f32, tag="s2", bufs=4, name=f"s2_{h}_{t}")
            for i in range(2):
                nc.scalar.activation(
                    probs[:, i * HF : (i + 1) * HF],
                    sc[i][:],
                    Exp,
                    scale=GAMMA,
                    accum_out=s2[:, i : i + 1],
                )
            nc.vector.tensor_tensor(
                sums[h][:, t : t + 1], s2[:, 0:1], s2[:, 1:2], op=mybir.AluOpType.add
            )
            nc.vector.reciprocal(rmat[h][:, t : t + 1], sums[h][:, t : t + 1])
            nc.vector.tensor_scalar_mul(probs[:], probs[:], rmat[h][:, t : t + 1])
            nc.sync.dma_start(attnw[h, t * P : (t + 1) * P, :], probs[:])

        def r_broadcast(h):
            """rmat[h] [128i, 16t] -> bf16 [128, 2048] row r[i] on every
            partition, via PE transpose + one-hot-row matmuls."""
            rT_ps = half(f"rT{h}")
            nc.tensor.transpose(rT_ps[:NT, :P], rmat[h][:], ident[:])
            rT_sb = sb.tile([NT, P], f32, tag="rT", bufs=2, name=f"rTs{h}")
            nc.vector.tensor_copy(rT_sb[:], rT_ps[:NT, :P])
            rbh = [half(f"rb{h}_{i}") for i in range(2)]
            for t in range(NT):
                nc.tensor.matmul(
                    rbh[t // 8][:, (t % 8) * P : (t % 8 + 1) * P],
                    lhsT=id16[:, t : t + 1].broadcast_to([NT, P]),
                    rhs=rT_sb[:],
                    start=True,
                    stop=True,
                )
            rbc = sb.tile([P, S], f32, tag="rbc", bufs=2, name=f"rbc{h}")
            for i in range(2):
                nc.vector.tensor_copy(rbc[:, i * HF : (i + 1) * HF], rbh[i][:])
            return rbc

        def b2_step(h, j, attq):
            """scores^T j-tile -> exp -> 4 attn@v matmuls into attq halves."""
            st = [half(f"st{h}_{j}_{i}") for i in range(2)]
            for i4 in range(4):
                nc.tensor.matmul(
                    st[i4 // 2][:, (i4 % 2) * 512 : (i4 % 2 + 1) * 512],
                    lhsT=kT[h][:, j * P : (j + 1) * P],
                    rhs=qT[h][:, i4 * 512 : (i4 + 1) * 512],
                    start=True,
                    stop=True,
                )
            expT = sb.tile([P, S], bf16, tag="xe", bufs=4, name=f"eT{h}_{j}")
            for i in range(2):
                nc.scalar.activation(
                    expT[:, i * HF : (i + 1) * HF], st[i][:], Exp, scale=GAMMA
                )
            r, jj = divmod(j, 8)
            for i4 in range(4):
                nc.tensor.matmul(
                    attq[i4 // 2][:, (i4 % 2) * 512 : (i4 % 2 + 1) * 512],
                    lhsT=vw[r][:, jj * DPC + h * P : jj * DPC + (h + 1) * P],
                    rhs=expT[:, i4 * 512 : (i4 + 1) * 512],
                    start=(j == 0),
                    stop=(j == NT - 1),
                )

        def att_evict(h, attq, rbc):
            a = sb.tile([P, S], bf16, tag="aT", bufs=2, name=f"aT{h}")
            for i in range(2):
                nc.vector.tensor_tensor(
                    a[:, i * HF : (i + 1) * HF],
                    attq[i][:],
                    rbc[:, i * HF : (i + 1) * HF],
                    op=mult,
                )
            return a

        # ---- phase B schedule ------------------------------------------
        # v rounds overlap head-0 B1 (v is PE-dense, B1 is ACT-dense).
        for t in range(NT):
            if t % 4 == 0:
                v_round(t // 4)
            b1_tile(0, t)
        rbc0 = r_broadcast(0)

        # head-0 attn@v interleaved with head-1 B1
        attq0 = [half("attq0_0"), half("attq0_1")]
        for j in range(NT):
            b2_step(0, j, attq0)
            b1_tile(1, j)
        aT0 = att_evict(0, attq0, rbc0)
        rbc1 = r_broadcast(1)

        attq1 = [half("attq1_0"), half("attq1_1")]
        for j in range(NT):
            b2_step(1, j, attq1)
        aT1 = att_evict(1, attq1, rbc1)
        aT = [aT0, aT1]

        # ---- phase C: o_proj (partial; host sums across cores) ---------
        for og in range(NT):
            oph = [half(f"op{og}_{i}") for i in range(2)]
            for i4 in range(4):
                for d in range(HPC):
                    nc.tensor.matmul(
                        oph[i4 // 2][:, (i4 % 2) * 512 : (i4 % 2 + 1) * 512],
                        lhsT=wo_sb[d][:, og * P : (og + 1) * P],
                        rhs=aT[d][:, i4 * 512 : (i4 + 1) * 512],
                        start=(d == 0),
                        stop=(d == HPC - 1),
                    )
            oev = sb.tile([P, S], f32, tag="probs", bufs=3, name=f"oev{og}")
            for i in range(2):
                if (og + i) % 2 == 0:
                    nc.scalar.copy(oev[:, i * HF : (i + 1) * HF], oph[i][:])
                else:
                    nc.vector.tensor_copy(oev[:, i * HF : (i + 1) * HF], oph[i][:])
            nc.sync.dma_start(outT[og * P : (og + 1) * P, :], oev[:])

    nc.compile()
    return nc


def _get_nc():
    if "nc" not in _CACHE:
        _install_ntff_hook()
        _CACHE["nc"] = _build_nc()
    return _CACHE["nc"]


def kernel(query_states, key_states, cos, sin, Wq, Wk, Wv, Wo, _trace=False):
    from concourse.bass_utils import run_bass_kernel_spmd

    nc = _get_nc()

    x = np.asarray(query_states, dtype=np.float32)[0]
    kx = np.asarray(key_states, dtype=np.float32)[0]
    cs = np.asarray(cos, dtype=np.float32)[0] + np.asarray(sin, dtype=np.float32)[0]
    xqT = x.T.astype(BF16)
    kxT_a = kx.T.astype(BF16)
    csT = np.ascontiguousarray(cs.T)
    Wq_, Wk_, Wv_, Wo_ = (np.asarray(w, dtype=np.float32) for w in (Wq, Wk, Wv, Wo))

    in_maps = []
    for c in range(N_CORES):
        sl = slice(c * DPC, (c + 1) * DPC)
        in_maps.append(
            {
                "xqT": xqT,
                "kxT": kxT_a,
                "csT": csT,
                "wkt": Wk_[sl, :].T.astype(BF16),
                "wqt": Wq_[sl, :].T.astype(BF16),
                "wvt": Wv_[sl, :].T.astype(BF16),
                "wot": Wo_[:, sl].T.astype(BF16),
            }
        )

    res = run_bass_kernel_spmd(nc, in_maps, list(range(N_CORES)), trace=_trace)
    _CACHE["last_results"] = res

    outT = res.results[0]["outT"].astype(np.float64)
    for c in range(1, N_CORES):
        outT += res.results[c]["outT"]
    out = np.ascontiguousarray(outT.T).astype(np.float32).reshape(1, S, H)
    attn = np.stack([res.results[c]["attnw"] for c in range(N_CORES)])
    attn = attn.reshape(1, NH, S, S)
    return out, attn
